# revision 13
# baseline (speedup 1.0000x reference)
"""Trainium2 Bass kernel for nn_HPool histogram_binning.

Math: z[n,c] = sum_hw tanh(x) * coeff[c, bin(x)] with 32 uniform bins over
[min(x), max(x)] (global min/max, computed host-side like the thresholds).

Scheme ("hinge + count stats at 4x"):
  T = tanh(x) (fp16, scalar engine, fused row-accum gives sum(T)).
  For each interior bin edge tau_j (j=1..31), with tt_j = tanh(tau_j):
    count stat  G_j = sum_f [T >= tt_j]          (one tensor_scalar, 4x mode)
    hinge stat  R_j = sum_f relu(T - tt_j)       (one tensor_scalar, 4x mode)
  Exact recovery: S_{>=j} := sum_f T*[T >= tt_j] = R_j + tt_j * G_j, and the
  per-bin tanh-mass S_b is a difference of adjacent S_{>=}.
  Tail trick: for bins fully outside |x| <= XCUT, tanh saturates so
  S_b ~= sign(bin) * cnt_b (error ~1e-3 of z); hinges are only emitted for
  the ~18 central edges. Counts are emitted for all 31 edges.
  z[r] is then a per-row linear mix of the ~50 raw stats with host-computed
  per-channel weights (single tensor_tensor mult + reduce per row-tile).

Cost: ~50 stats/element instead of 32 full passes; DVE tensor_scalar with
immediate scalars + accum_out runs in 4x perf mode (0.25 cyc/elem, fp16),
with ~11 count stats offloaded to the scalar engine (Sign+bias+accum) to
balance ACT (tanh pass) and DVE.

Sharding: data-parallel over N across 8 cores (8 samples each).
"""

import os
import numpy as np

N, C, H, W, BINS = 64, 64, 128, 128, 32
HW = H * W
NCORES = 8
NPC = N // NCORES          # samples per core
ROWS = NPC * C             # 512 rows per core, row r = n_local*C + c
P = 128
NT = ROWS // P             # 4 row-tiles
F = 8192                   # free-dim chunk (half a row-tile)
NF = HW // F               # 2 chunks per row-tile

XCUT = float(os.environ.get("KERNEL_XCUT", "3.0"))   # hinge edges kept where |tau| <= XCUT
N_ACT = int(os.environ.get("KERNEL_NACT", "11"))     # count stats on scalar engine

LAST_EXEC_NS = None
_CACHE = {}


def _edge_info(gmin: float, gmax: float):
    """Edges tau_1..tau_31, tanh thresholds, hinge set, ACT/DVE count split."""
    step = (np.float64(gmax) - np.float64(gmin)) / np.float64(BINS)
    edges = (np.float64(gmin) + step * np.arange(1, BINS)).astype(np.float64)
    tt = np.tanh(edges)
    jh = [j for j in range(BINS - 1) if abs(edges[j]) <= XCUT]
    assert jh and jh == list(range(jh[0], jh[-1] + 1)), "hinge edges not contiguous"
    act_j = list(range(min(N_ACT, BINS - 1)))        # count edges on ACT (Sign)
    return edges, tt, jh, set(act_j)


def _stat_cols(jh):
    """Column layout inside the [P, 64] stats tile.

    col 0 / col 62: sum(T) of the two DMA halves; col 63: const 1.
    """
    rcol = {j: 1 + i for i, j in enumerate(jh)}           # max-hinge stats
    g0 = 1 + len(jh)
    gcol = {j: g0 + j for j in range(BINS - 1)}           # count stats
    assert g0 + BINS - 1 <= 61
    return rcol, gcol


def _host_weights(coeff: np.ndarray, gmin: float, gmax: float):
    """Per-channel mixing weights over the raw stat columns (fp64 -> fp32)."""
    edges, tt, jh, act_j = _edge_info(gmin, gmax)
    rcol, gcol = _stat_cols(jh)
    jhset = set(jh)
    tau_lo = np.float64(gmin) + (np.float64(gmax) - np.float64(gmin)) / BINS * np.arange(BINS)

    w = np.zeros((C, 64), dtype=np.float64)
    const = np.zeros(C, dtype=np.float64)

    def add_g(j, v):
        if j in act_j:   # raw stat is sum(sign(T-tt)) = 2G - n
            w[:, gcol[j]] += v / 2.0
            const[:] += v * (HW / 2.0)
        else:            # raw stat is G directly
            w[:, gcol[j]] += v

    def add_s_geq(e, v):
        # S_{>=e} = M_j + tt_j*G_j - tt_j*n  (M_j = sum max(T, tt_j))
        if e == 0:
            w[:, 0] += v                     # sum(T)
        elif e < BINS:
            j = e - 1
            w[:, rcol[j]] += v
            add_g(j, v * tt[j])
            const[:] += -v * tt[j] * HW
        # e == BINS: zero

    def add_g_geq(e, v):
        if e == 0:
            const[:] += v * HW
        elif e < BINS:
            add_g(e - 1, v)

    for b in range(BINS):
        wb = coeff[:, b].astype(np.float64)
        lo_ok = (b == 0) or (b - 1) in jhset
        hi_ok = (b == BINS - 1) or b in jhset
        if lo_ok and hi_ok:
            add_s_geq(b, wb)
            add_s_geq(b + 1, -wb)
        else:
            sgn = 1.0 if tau_lo[b] >= 0 else -1.0
            add_g_geq(b, wb * sgn)
            add_g_geq(b + 1, -wb * sgn)

    w[:, 62] = w[:, 0]      # sum(T) of second DMA half, same weight
    w[:, 63] = const
    return w.astype(np.float32)


def _new_nc():
    import concourse.bacc as bacc

    return bacc.Bacc(
        "TRN2", target_bir_lowering=False, debug=False, num_devices=NCORES
    )


def _build_main(gmin: float, gmax: float):
    import concourse.mybir as mybir
    from concourse.tile import TileContext

    fp32 = mybir.dt.float32
    fp16 = mybir.dt.float16
    AX = mybir.AxisListType.X
    OP = mybir.AluOpType
    AF = mybir.ActivationFunctionType

    edges, tt, jh, act_j = _edge_info(gmin, gmax)
    rcol, gcol = _stat_cols(jh)
    dve_count_j = [j for j in range(BINS - 1) if j not in act_j]

    nc = _new_nc()
    xs = nc.dram_tensor("xs", [ROWS, HW], fp32, kind="ExternalInput")
    wt = nc.dram_tensor("wt", [P, 64], fp32, kind="ExternalInput")
    bs = nc.dram_tensor("bs", [P, max(len(act_j), 1)], fp32, kind="ExternalInput")
    z = nc.dram_tensor("z", [ROWS, 1], fp32, kind="ExternalOutput")

    with TileContext(nc, num_cores=NCORES) as tc:
        with (
            tc.tile_pool(name="xp", bufs=2) as xp,
            tc.tile_pool(name="tp", bufs=2) as tp,
            tc.tile_pool(name="scr", bufs=1) as scr,
            tc.tile_pool(name="sp", bufs=2) as sp,
            tc.tile_pool(name="stat", bufs=1) as stat,
        ):
            wts = stat.tile([P, 64], fp32, tag="wts")
            nc.sync.dma_start(out=wts[:], in_=wt[:, :])
            bss = stat.tile([P, max(len(act_j), 1)], fp32, tag="bss")
            nc.sync.dma_start(out=bss[:], in_=bs[:, :])

            for t in range(NT):
                V = sp.tile([P, 64], fp32, tag="V")
                nc.vector.memset(V[:], 0.0)
                nc.vector.memset(V[:, 63:64], 1.0)
                T = tp.tile([P, HW], fp16, tag="T")
                for h in range(NF):
                    X = xp.tile([P, F], fp32, tag="X")
                    nc.sync.dma_start(
                        out=X[:], in_=xs[t * P:(t + 1) * P, h * F:(h + 1) * F]
                    )
                    stc = 0 if h == 0 else 62
                    nc.scalar.activation(
                        out=T[:, h * F:(h + 1) * F], in_=X[:], func=AF.Tanh,
                        accum_out=V[:, stc:stc + 1],
                    )
                SA = scr.tile([P, HW], fp16, tag="SA")
                for i, j in enumerate(sorted(act_j)):
                    nc.scalar.activation(
                        out=SA[:], in_=T[:], func=AF.Sign,
                        bias=bss[:, i:i + 1],
                        accum_out=V[:, gcol[j]:gcol[j] + 1],
                    )
                # With accum_out, op1 is the REDUCTION op: accum = reduce_op1(op0(in, s1)).
                SD = scr.tile([P, HW], fp16, tag="SD")
                for j in jh:
                    nc.vector.tensor_scalar(
                        out=SD[:], in0=T[:],
                        scalar1=float(tt[j]), scalar2=0.0,
                        op0=OP.max, op1=OP.add,
                        accum_out=V[:, rcol[j]:rcol[j] + 1],
                    )
                for j in dve_count_j:
                    nc.vector.tensor_scalar(
                        out=SD[:], in0=T[:],
                        scalar1=float(tt[j]), scalar2=0.0,
                        op0=OP.is_ge, op1=OP.add,
                        accum_out=V[:, gcol[j]:gcol[j] + 1],
                    )
                ZC = sp.tile([P, 64], fp32, tag="ZC")
                nc.vector.tensor_tensor(out=ZC[:], in0=V[:], in1=wts[:], op=OP.mult)
                zcol = sp.tile([P, 1], fp32, tag="zcol")
                nc.vector.tensor_reduce(out=zcol[:], in_=ZC[:], axis=AX, op=OP.add)
                nc.sync.dma_start(out=z[t * P:(t + 1) * P, :], in_=zcol[:])
    nc.compile()
    return nc


def _prep_in_maps(x: np.ndarray, coeff: np.ndarray, gmin: float, gmax: float):
    wt = _host_weights(coeff, gmin, gmax)                 # [C, 64]
    wt128 = np.ascontiguousarray(wt[np.arange(P) % C])    # row r -> channel r%64

    _, tt, _, act_j = _edge_info(gmin, gmax)
    aj = sorted(act_j)
    nbias = max(len(aj), 1)
    bs128 = np.zeros((P, nbias), dtype=np.float32)
    for i, j in enumerate(aj):
        bs128[:, i] = np.float32(-tt[j])

    xr = x.reshape(N, C, HW)
    in_maps = []
    for k in range(NCORES):
        shard = np.ascontiguousarray(
            xr[k * NPC:(k + 1) * NPC].reshape(ROWS, HW), dtype=np.float32
        )
        in_maps.append({"xs": shard, "wt": wt128, "bs": bs128})
    return in_maps


def kernel(x: np.ndarray, coeff: np.ndarray) -> np.ndarray:
    global LAST_EXEC_NS
    from concourse.bass_utils import run_bass_kernel_spmd

    x = np.asarray(x, dtype=np.float32)
    coeff = np.asarray(coeff, dtype=np.float32)

    gmin = float(x.min())
    gmax = float(x.max())

    key = ("nc", gmin, gmax)
    if key not in _CACHE:
        _CACHE[key] = _build_main(gmin, gmax)
    nc = _CACHE[key]
    _CACHE["nc"] = nc   # test.py reads _CACHE["nc"] for the cost-model timeline

    in_maps = _prep_in_maps(x, coeff, gmin, gmax)

    trace = bool(os.environ.get("KERNEL_TRACE"))
    res = run_bass_kernel_spmd(
        nc, in_maps, list(range(NCORES)), trace=trace,
    )
    LAST_EXEC_NS = res.exec_time_ns

    out = np.empty((N, C), dtype=np.float32)
    for k in range(NCORES):
        out[k * NPC:(k + 1) * NPC] = res.results[k]["z"].reshape(NPC, C)
    return out


# revision 16
# speedup vs baseline: 1.0061x; 1.0061x over previous
"""Trainium2 Bass kernel for nn_HPool histogram_binning.

Math: z[n,c] = sum_hw tanh(x) * coeff[c, bin(x)] with 32 uniform bins over
[min(x), max(x)] (global min/max, computed host-side like the thresholds).

Scheme ("hinge + count stats at 4x"):
  T = tanh(x) (fp16, scalar engine, fused row-accum gives sum(T)).
  For each interior bin edge tau_j (j=1..31), with tt_j = tanh(tau_j):
    count stat  G_j = sum_f [T >= tt_j]          (one tensor_scalar, 4x mode)
    hinge stat  R_j = sum_f relu(T - tt_j)       (one tensor_scalar, 4x mode)
  Exact recovery: S_{>=j} := sum_f T*[T >= tt_j] = R_j + tt_j * G_j, and the
  per-bin tanh-mass S_b is a difference of adjacent S_{>=}.
  Tail trick: for bins fully outside |x| <= XCUT, tanh saturates so
  S_b ~= sign(bin) * cnt_b (error ~1e-3 of z); hinges are only emitted for
  the ~18 central edges. Counts are emitted for all 31 edges.
  z[r] is then a per-row linear mix of the ~50 raw stats with host-computed
  per-channel weights (single tensor_tensor mult + reduce per row-tile).

Cost: ~50 stats/element instead of 32 full passes; DVE tensor_scalar with
immediate scalars + accum_out runs in 4x perf mode (0.25 cyc/elem, fp16),
with ~11 count stats offloaded to the scalar engine (Sign+bias+accum) to
balance ACT (tanh pass) and DVE.

Sharding: data-parallel over N across 8 cores (8 samples each).
"""

import os
import numpy as np

N, C, H, W, BINS = 64, 64, 128, 128, 32
HW = H * W
NCORES = 8
NPC = N // NCORES          # samples per core
ROWS = NPC * C             # 512 rows per core, row r = n_local*C + c
P = 128
NT = ROWS // P             # 4 row-tiles
F = 8192                   # free-dim chunk (half a row-tile)
NF = HW // F               # 2 chunks per row-tile

XCUT = float(os.environ.get("KERNEL_XCUT", "3.0"))   # hinge edges kept where |tau| <= XCUT
N_ACT = int(os.environ.get("KERNEL_NACT", "11"))     # count stats on scalar engine

LAST_EXEC_NS = None
_CACHE = {}


def _edge_info(gmin: float, gmax: float):
    """Edges tau_1..tau_31, tanh thresholds, hinge set, ACT/DVE count split."""
    step = (np.float64(gmax) - np.float64(gmin)) / np.float64(BINS)
    edges = (np.float64(gmin) + step * np.arange(1, BINS)).astype(np.float64)
    tt = np.tanh(edges)
    jh = [j for j in range(BINS - 1) if abs(edges[j]) <= XCUT]
    assert jh and jh == list(range(jh[0], jh[-1] + 1)), "hinge edges not contiguous"
    act_j = list(range(min(N_ACT, BINS - 1)))        # count edges on ACT (Sign)
    return edges, tt, jh, set(act_j)


def _stat_cols(jh):
    """Column layout inside the [P, 64] stats tile.

    col 0 / col 62: sum(T) of the two DMA halves; col 63: const 1.
    """
    rcol = {j: 1 + i for i, j in enumerate(jh)}           # max-hinge stats
    g0 = 1 + len(jh)
    gcol = {j: g0 + j for j in range(BINS - 1)}           # count stats
    assert g0 + BINS - 1 <= 61
    return rcol, gcol


def _host_weights(coeff: np.ndarray, gmin: float, gmax: float):
    """Per-channel mixing weights over the raw stat columns (fp64 -> fp32)."""
    edges, tt, jh, act_j = _edge_info(gmin, gmax)
    rcol, gcol = _stat_cols(jh)
    jhset = set(jh)
    tau_lo = np.float64(gmin) + (np.float64(gmax) - np.float64(gmin)) / BINS * np.arange(BINS)

    w = np.zeros((C, 64), dtype=np.float64)
    const = np.zeros(C, dtype=np.float64)

    def add_g(j, v):
        if j in act_j:   # raw stat is sum(sign(T-tt)) = 2G - n
            w[:, gcol[j]] += v / 2.0
            const[:] += v * (HW / 2.0)
        else:            # raw stat is G directly
            w[:, gcol[j]] += v

    def add_s_geq(e, v):
        # S_{>=e} = M_j + tt_j*G_j - tt_j*n  (M_j = sum max(T, tt_j))
        if e == 0:
            w[:, 0] += v                     # sum(T)
        elif e < BINS:
            j = e - 1
            w[:, rcol[j]] += v
            add_g(j, v * tt[j])
            const[:] += -v * tt[j] * HW
        # e == BINS: zero

    def add_g_geq(e, v):
        if e == 0:
            const[:] += v * HW
        elif e < BINS:
            add_g(e - 1, v)

    for b in range(BINS):
        wb = coeff[:, b].astype(np.float64)
        lo_ok = (b == 0) or (b - 1) in jhset
        hi_ok = (b == BINS - 1) or b in jhset
        if lo_ok and hi_ok:
            add_s_geq(b, wb)
            add_s_geq(b + 1, -wb)
        else:
            sgn = 1.0 if tau_lo[b] >= 0 else -1.0
            add_g_geq(b, wb * sgn)
            add_g_geq(b + 1, -wb * sgn)

    w[:, 63] = const
    return w.astype(np.float32)


def _new_nc():
    import concourse.bacc as bacc

    return bacc.Bacc(
        "TRN2", target_bir_lowering=False, debug=False, num_devices=NCORES
    )


def _build_main(gmin: float, gmax: float):
    import concourse.mybir as mybir
    from concourse.tile import TileContext

    fp32 = mybir.dt.float32
    fp16 = mybir.dt.float16
    AX = mybir.AxisListType.X
    OP = mybir.AluOpType
    AF = mybir.ActivationFunctionType

    edges, tt, jh, act_j = _edge_info(gmin, gmax)
    rcol, gcol = _stat_cols(jh)
    dve_count_j = [j for j in range(BINS - 1) if j not in act_j]

    nc = _new_nc()
    xs = nc.dram_tensor("xs", [ROWS, HW], fp32, kind="ExternalInput")
    wt = nc.dram_tensor("wt", [P, 64], fp32, kind="ExternalInput")
    bs = nc.dram_tensor("bs", [P, max(len(act_j), 1)], fp32, kind="ExternalInput")
    z = nc.dram_tensor("z", [ROWS, 1], fp32, kind="ExternalOutput")

    with TileContext(nc, num_cores=NCORES) as tc:
        with (
            tc.tile_pool(name="xp", bufs=2) as xp,
            tc.tile_pool(name="tp", bufs=2) as tp,
            tc.tile_pool(name="scr", bufs=1) as scr,
            tc.tile_pool(name="sp", bufs=2) as sp,
            tc.tile_pool(name="stat", bufs=1) as stat,
        ):
            wts = stat.tile([P, 64], fp32, tag="wts")
            nc.sync.dma_start(out=wts[:], in_=wt[:, :])
            bss = stat.tile([P, max(len(act_j), 1)], fp32, tag="bss")
            nc.sync.dma_start(out=bss[:], in_=bs[:, :])

            for t in range(NT):
                V = sp.tile([P, 128], fp32, tag="V")
                nc.vector.memset(V[:], 0.0)
                for h in range(NF):
                    off = 64 * h
                    X = xp.tile([P, F], fp32, tag="X")
                    nc.sync.dma_start(
                        out=X[:], in_=xs[t * P:(t + 1) * P, h * F:(h + 1) * F]
                    )
                    T = tp.tile([P, F], fp16, tag="T")
                    nc.scalar.activation(
                        out=T[:], in_=X[:], func=AF.Tanh,
                        accum_out=V[:, off:off + 1],
                    )
                    # ACT count stats read X directly (x-domain thresholds):
                    # exact fp32 counts, and no dependency on the tanh pass.
                    SA = scr.tile([P, F], fp16, tag="SA")
                    for i, j in enumerate(sorted(act_j)):
                        nc.scalar.activation(
                            out=SA[:], in_=X[:], func=AF.Sign,
                            bias=bss[:, i:i + 1],
                            accum_out=V[:, off + gcol[j]:off + gcol[j] + 1],
                        )
                    # With accum_out, op1 is the REDUCTION op:
                    # accum = reduce_op1(op0(in, s1)).
                    SD = scr.tile([P, F], fp16, tag="SD")
                    for j in jh:
                        nc.vector.tensor_scalar(
                            out=SD[:], in0=T[:],
                            scalar1=float(tt[j]), scalar2=0.0,
                            op0=OP.max, op1=OP.add,
                            accum_out=V[:, off + rcol[j]:off + rcol[j] + 1],
                        )
                    for j in dve_count_j:
                        nc.vector.tensor_scalar(
                            out=SD[:], in0=T[:],
                            scalar1=float(tt[j]), scalar2=0.0,
                            op0=OP.is_ge, op1=OP.add,
                            accum_out=V[:, off + gcol[j]:off + gcol[j] + 1],
                        )
                Vs = sp.tile([P, 64], fp32, tag="Vs")
                nc.vector.tensor_tensor(
                    out=Vs[:], in0=V[:, 0:64], in1=V[:, 64:128], op=OP.add
                )
                nc.vector.memset(Vs[:, 63:64], 1.0)
                ZC = sp.tile([P, 64], fp32, tag="ZC")
                nc.vector.tensor_tensor(out=ZC[:], in0=Vs[:], in1=wts[:], op=OP.mult)
                zcol = sp.tile([P, 1], fp32, tag="zcol")
                nc.vector.tensor_reduce(out=zcol[:], in_=ZC[:], axis=AX, op=OP.add)
                nc.sync.dma_start(out=z[t * P:(t + 1) * P, :], in_=zcol[:])
    nc.compile()
    return nc


def _prep_in_maps(x: np.ndarray, coeff: np.ndarray, gmin: float, gmax: float):
    wt = _host_weights(coeff, gmin, gmax)                 # [C, 64]
    wt128 = np.ascontiguousarray(wt[np.arange(P) % C])    # row r -> channel r%64

    edges, _, _, act_j = _edge_info(gmin, gmax)
    aj = sorted(act_j)
    nbias = max(len(aj), 1)
    bs128 = np.zeros((P, nbias), dtype=np.float32)
    for i, j in enumerate(aj):
        bs128[:, i] = np.float32(-edges[j])   # ACT Sign reads X: x-domain bias

    xr = x.reshape(N, C, HW)
    in_maps = []
    for k in range(NCORES):
        shard = np.ascontiguousarray(
            xr[k * NPC:(k + 1) * NPC].reshape(ROWS, HW), dtype=np.float32
        )
        in_maps.append({"xs": shard, "wt": wt128, "bs": bs128})
    return in_maps


def kernel(x: np.ndarray, coeff: np.ndarray) -> np.ndarray:
    global LAST_EXEC_NS
    from concourse.bass_utils import run_bass_kernel_spmd

    x = np.asarray(x, dtype=np.float32)
    coeff = np.asarray(coeff, dtype=np.float32)

    gmin = float(x.min())
    gmax = float(x.max())

    key = ("nc", gmin, gmax)
    if key not in _CACHE:
        _CACHE[key] = _build_main(gmin, gmax)
    nc = _CACHE[key]
    _CACHE["nc"] = nc   # test.py reads _CACHE["nc"] for the cost-model timeline

    in_maps = _prep_in_maps(x, coeff, gmin, gmax)

    trace = bool(os.environ.get("KERNEL_TRACE"))
    res = run_bass_kernel_spmd(
        nc, in_maps, list(range(NCORES)), trace=trace,
    )
    LAST_EXEC_NS = res.exec_time_ns

    out = np.empty((N, C), dtype=np.float32)
    for k in range(NCORES):
        out[k * NPC:(k + 1) * NPC] = res.results[k]["z"].reshape(NPC, C)
    return out
